# revision 47
# baseline (speedup 1.0000x reference)
"""LCAOConv message-passing kernel for 8 Trainium2 NeuronCores (v3).

Strategy (edge-parallel, owner = src core, degree-sorted node-per-partition):
  - Node shard: core k owns nodes [k*NSH, (k+1)*NSH).  Within each core,
    nodes are permuted by (degree, #dsts-in-lower-half) descending so that
    each 128-node chunk has near-uniform slot counts.
  - Phase A: each core computes h = MLP(x), c = MLP(coeffs) for its shard
    (inputs pre-silu'd on host, all matmuls bf16), writes a fused row table
    T[n] = [c[n] (R*D), h[n] (D), pad] in bf16 (row padded to 768 B), then
    AllGather -> full table on every core.
  - Phase B: chunk ch holds 128 src nodes, one per partition; row p carries
    the edge slots of its node along the free axis.  T[dst] rows for all
    128*L slots arrive via gpsimd.dma_gather (one SWDGE call per table
    half; int16 indices, dst core < 4 -> lower half).  c_src is the
    partition-local row broadcast along the slot axis, so the per-edge
    reweighting and both l2-normalizations are wide DVE ops and the
    segment-sum is a free-axis reduction.  Final agg @ Wu via PE.
"""

import sys
for _p in ("/opt/trn_rl_repo", "/root/.axon_site/_ro/trn_rl_repo"):
    if _p not in sys.path:
        sys.path.insert(0, _p)

import numpy as np
from ml_dtypes import bfloat16

import concourse.bass as bass
import concourse.bacc as bacc
import concourse.mybir as mybir
import concourse.tile as tile
from concourse.bass_utils import run_bass_kernel_spmd
from concourse.library_config import mlp as mlp_lib
from concourse.masks import make_identity

F32 = mybir.dt.float32
BF16 = mybir.dt.bfloat16
I16 = mybir.dt.int16

NC = 8          # cores
P = 128         # partitions
XW = 1024       # phase-A tile width (node MLP)
XW2 = 1024      # phase-A tile width (coeffs MLP)
TDP = 384       # padded table row (384 bf16 = 768 B, 256 B-multiple)
LSEG = 26       # max slot columns per processing segment


def _build(NSH, H, D, C, R, LAs, LBs):
    """Build the Bass program (identical on all cores)."""
    N = NSH * NC
    NH2 = N // 2
    CD = R * D                # c part of a table row
    TD = CD + D               # used part of a row
    assert TD <= TDP
    n_chunks = (NSH + P - 1) // P
    assert len(LAs) == len(LBs) == n_chunks
    NR = NSH * R
    Ls = [a + b for a, b in zip(LAs, LBs)]
    SLOT = int(sum(Ls))
    offs = np.concatenate(([0], np.cumsum(Ls))).astype(int)

    nc = bacc.Bacc("TRN2", num_devices=NC, num_swdge_queues=4)

    # ---- I/O ----
    sxT = nc.dram_tensor("sxT", [H, NSH], BF16, kind="ExternalInput")
    scT = nc.dram_tensor("scT", [C, NR], BF16, kind="ExternalInput")
    W1b = nc.dram_tensor("W1b", [H, H], BF16, kind="ExternalInput")
    b1 = nc.dram_tensor("b1", [H, 1], F32, kind="ExternalInput")
    W2b = nc.dram_tensor("W2b", [H, D], BF16, kind="ExternalInput")
    b2r = nc.dram_tensor("b2r", [P, 8 * D], F32, kind="ExternalInput")
    Wc1b = nc.dram_tensor("Wc1b", [C, H], BF16, kind="ExternalInput")
    Wc2b = nc.dram_tensor("Wc2b", [H, D], BF16, kind="ExternalInput")
    Wub = nc.dram_tensor("Wub", [D, H], BF16, kind="ExternalInput")
    idxW = nc.dram_tensor("idxW", [P, 8 * SLOT], I16, kind="ExternalInput")
    rbfE = nc.dram_tensor("rbfE", [P, SLOT * R * 2], BF16,
                          kind="ExternalInput")
    out = nc.dram_tensor("out", [NSH, H], F32, kind="ExternalOutput")

    # ---- internal DRAM ----
    T_loc = nc.dram_tensor("T_loc", [NSH, TDP], BF16, kind="Internal")
    T_full = nc.dram_tensor("T_full", [N, TDP], BF16, kind="Internal",
                            addr_space="Shared")

    with tile.TileContext(nc) as tc:
        with (
            tc.tile_pool(name="const", bufs=1) as cpool,
            tc.tile_pool(name="a_in", bufs=2) as a_in,
            tc.tile_pool(name="a_mid", bufs=3) as a_mid,
            tc.tile_pool(name="a_out", bufs=4) as a_out,
            tc.tile_pool(name="b_gat", bufs=3) as b_gat,
            tc.tile_pool(name="b_loc", bufs=4) as b_loc,
            tc.tile_pool(name="b_big", bufs=2) as b_big,
            tc.tile_pool(name="b_one", bufs=1) as b_one,
            tc.tile_pool(name="b_sm", bufs=1) as b_sm,
            tc.tile_pool(name="b_out", bufs=2) as b_out,
        ):
            # ---------- constants ----------
            W1_s = cpool.tile([H, H], BF16)
            nc.sync.dma_start(W1_s[:], W1b[:])
            b1_s = cpool.tile([H, 1], F32)
            nc.sync.dma_start(b1_s[:], b1[:])
            W2_s = cpool.tile([H, D], BF16)
            nc.sync.dma_start(W2_s[:], W2b[:])
            b2_s = cpool.tile([P, 8 * D], F32)
            nc.sync.dma_start(b2_s[:], b2r[:])
            Wc1_s = cpool.tile([C, H], BF16)
            nc.sync.dma_start(Wc1_s[:], Wc1b[:])
            Wc2_s = cpool.tile([H, D], BF16)
            nc.sync.dma_start(Wc2_s[:], Wc2b[:])
            Wu_s = cpool.tile([D, H], BF16)
            nc.sync.dma_start(Wu_s[:], Wub[:])
            ident = cpool.tile([P, P], BF16)
            make_identity(nc, ident[:])
            eps_s = cpool.tile([P, 1], F32)
            nc.vector.memset(eps_s[:], 1e-24)

            # edge metadata, resident in SBUF (rbf loaded per chunk)
            idx_s = cpool.tile([P, 8 * SLOT], I16)
            nc.sync.dma_start(idx_s[:], idxW[:])

            # ---------- phase A: node MLP -> T_loc h columns ----------
            with (
                tc.tile_pool(name="a_ps", bufs=2, space="PSUM") as a_ps,
                tc.tile_pool(name="a_ps2", bufs=2, space="PSUM") as a_ps2,
            ):
                nxt = (NSH + XW - 1) // XW
                for j in range(nxt):
                    w = min(XW, NSH - j * XW)
                    xt = a_in.tile([H, XW], BF16, tag="xt")
                    nc.sync.dma_start(xt[:, :w], sxT[:, j * XW:j * XW + w])
                    h1p = a_ps.tile([H, XW], F32, tag="h1p")
                    for hh in range(0, w, 512):
                        hw = min(512, w - hh)
                        nc.tensor.matmul(h1p[:, hh:hh + hw], lhsT=W1_s[:],
                                         rhs=xt[:, hh:hh + hw],
                                         start=True, stop=True)
                    sh1 = a_mid.tile([H, XW], BF16, tag="sh1")
                    nc.scalar.activation(sh1[:, :w], h1p[:, :w],
                                         mybir.ActivationFunctionType.Silu,
                                         bias=b1_s[:])
                    nb = (w + P - 1) // P
                    h2p = a_ps2.tile([P, 8, D], F32, tag="h2p")
                    for b in range(nb):
                        bw = min(P, w - b * P)
                        nc.tensor.matmul(h2p[:bw, b, :],
                                         lhsT=sh1[:, b * P:b * P + bw],
                                         rhs=W2_s[:], start=True, stop=True)
                    h2r = a_out.tile([P, 8, D], BF16, tag="h2r")
                    if w == XW:
                        nc.vector.tensor_add(
                            h2r[:], h2p[:],
                            b2_s[:].rearrange("p (b d) -> p b d", d=D))
                        n0 = j * XW
                        nc.sync.dma_start(
                            T_loc[n0:n0 + XW, CD:TD].rearrange(
                                "(b p) d -> p b d", p=P),
                            h2r[:])
                    else:
                        for b in range(nb):
                            bw = min(P, w - b * P)
                            nc.vector.tensor_add(h2r[:bw, b, :], h2p[:bw, b, :],
                                                 b2_s[:bw, b * D:(b + 1) * D])
                            n0 = j * XW + b * P
                            nc.sync.dma_start(T_loc[n0:n0 + bw, CD:TD],
                                              h2r[:bw, b, :])

            # ---- phase A: coeffs MLP -> T_loc c columns ----
            # scT columns are (r, n)-major (host side), so each tile writes
            # one r-slice of T_loc's c columns directly; skipping the old
            # Cstage round-trip lets the chunked AllGather overlap the rest
            # of the coeffs MLP.
            with (
                tc.tile_pool(name="c_ps", bufs=2, space="PSUM") as a_ps,
                tc.tile_pool(name="c_ps2", bufs=2, space="PSUM") as a_ps2,
            ):
                for r in range(R):
                    base = r * NSH
                    for j0 in range(0, NSH, XW2):
                        w = min(XW2, NSH - j0)
                        ct = a_in.tile([C, XW2], BF16, tag="ct")
                        nc.sync.dma_start(ct[:, :w],
                                          scT[:, base + j0:base + j0 + w])
                        c1p = a_ps.tile([H, XW2], F32, tag="c1p")
                        for hh in range(0, w, 512):
                            hw = min(512, w - hh)
                            nc.tensor.matmul(c1p[:, hh:hh + hw], lhsT=Wc1_s[:],
                                             rhs=ct[:, hh:hh + hw],
                                             start=True, stop=True)
                        sc1 = a_mid.tile([H, XW2], BF16, tag="sc1")
                        nc.scalar.activation(sc1[:, :w], c1p[:, :w],
                                             mybir.ActivationFunctionType.Silu)
                        nb = (w + P - 1) // P
                        c2p = a_ps2.tile([P, R, D], F32, tag="c2p")
                        for b in range(nb):
                            bw = min(P, w - b * P)
                            nc.tensor.matmul(c2p[:bw, b, :],
                                             lhsT=sc1[:, b * P:b * P + bw],
                                             rhs=Wc2_s[:], start=True,
                                             stop=True)
                        c2r = a_out.tile([P, R, D], BF16, tag="c2r")
                        if w == XW2:
                            nc.vector.tensor_copy(c2r[:], c2p[:])
                            nc.sync.dma_start(
                                T_loc[j0:j0 + w, r * D:(r + 1) * D].rearrange(
                                    "(b p) d -> p b d", p=P),
                                c2r[:])
                        else:
                            for b in range(nb):
                                bw = min(P, w - b * P)
                                nc.vector.tensor_copy(c2r[:bw, b, :],
                                                      c2p[:bw, b, :])
                                n0 = j0 + b * P
                                nc.sync.dma_start(
                                    T_loc[n0:n0 + bw, r * D:(r + 1) * D],
                                    c2r[:bw, b, :])

            # ---------- AllGather the table ----------
            nc.gpsimd.collective_compute(
                "AllGather",
                mybir.AluOpType.bypass,
                replica_groups=[list(range(NC))],
                ins=[T_loc[:]],
                outs=[T_full[:]],
            )

            import os
            dump_T = os.environ.get("KERNEL_DEBUG_TDUMP") == "1"
            if dump_T:
                Tdump = nc.dram_tensor("Tdump", [N, TDP], BF16,
                                       kind="ExternalOutput")
                with tc.tile_pool(name="dbg", bufs=2) as dbg:
                    for jj in range(0, N, P):
                        ww = min(P, N - jj)
                        tt = dbg.tile([P, TDP], BF16, tag="tt")
                        nc.sync.dma_start(tt[:ww, :], T_full[jj:jj + ww, :])
                        nc.sync.dma_start(Tdump[jj:jj + ww, :], tt[:ww, :])

            # ---------- phase B ----------
            with (
                tc.tile_pool(name="b_ps_t", bufs=2, space="PSUM") as b_ps_t,
                tc.tile_pool(name="b_ps_f", bufs=2, space="PSUM") as b_ps_f,
            ):
                gq = [0]  # alternate SWDGE queue (Q7 core pair) per call
                for ch in range(n_chunks):
                    wn = min(P, NSH - ch * P)
                    LA, LB = int(LAs[ch]), int(LBs[ch])
                    L = LA + LB
                    off = int(offs[ch])
                    nseg = (L + LSEG - 1) // LSEG

                    # local c block (+1)
                    cloc = b_loc.tile([P, CD], BF16, tag="cloc")
                    if wn < P:
                        nc.vector.memset(cloc[:], 0.0)
                    nc.sync.dma_start(cloc[:wn, :],
                                      T_loc[ch * P:ch * P + wn, 0:CD])
                    cp1 = b_loc.tile([P, CD], BF16, tag="cp1")
                    nc.vector.tensor_scalar_add(cp1[:], cloc[:], 1.0)

                    # rbf pairs for this chunk [P, L, R, 2]
                    rbf2 = b_loc.tile([P, L * R * 2], BF16, tag="rbf2")
                    nc.sync.dma_start(
                        rbf2[:], rbfE[:, off * R * 2:(off + L) * R * 2])

                    accf = None
                    if nseg > 1:
                        accf = b_out.tile([P, D], BF16, tag="accf")

                    for s in range(nseg):
                        c0 = s * LSEG
                        c1 = min(L, c0 + LSEG)
                        Lg = c1 - c0

                        # gather T[dst]: one dma_gather per (segment, table
                        # half) — descriptor rings hold 1024 descs/engine, a
                        # 26-column call needs 209, so no ring pressure.
                        # dma_gather per (segment, table half) in runs of <=8
                        # slot columns: 1024 indices is the descriptor-ring
                        # limit per call (bigger calls wedge the device)
                        td = b_gat.tile([P, Lg, TDP], BF16, tag="td")
                        GW = 8
                        spans = []
                        if c0 < LA:
                            a1 = min(c1, LA)
                            for g0 in range(c0, a1, GW):
                                spans.append((g0, min(g0 + GW, a1),
                                              T_full[0:NH2, :]))
                        if c1 > LA:
                            b0 = max(c0, LA)
                            for g0 in range(b0, c1, GW):
                                spans.append((g0, min(g0 + GW, c1),
                                              T_full[NH2:N, :]))
                        for g0, g1, src in spans:
                            nw = 128 * (g1 - g0)
                            nc.gpsimd.dma_gather(
                                td[:, g0 - c0:g1 - c0, :], src,
                                idx_s[:, 8 * (off + g0):8 * (off + g1)],
                                nw, nw, TDP, queue_num=gq[0] & 3)
                            gq[0] += 1

                        # ce = c[dst] * (c[src]+1)   [P, Lg, CD]
                        ce = b_big.tile([P, Lg * CD], BF16, tag="ce")
                        nc.vector.tensor_tensor(
                            out=ce.rearrange("p (l c) -> p l c", c=CD),
                            in0=cp1[:].rearrange("p (o c) -> p o c", o=1)
                                .to_broadcast([P, Lg, CD]),
                            in1=td[:, :, 0:CD],
                            op=mybir.AluOpType.mult)

                        # q = sum_d ce^2 per (l, r)  (square on ACT engine)
                        sq = b_big.tile([P, Lg * CD], BF16, tag="sq")
                        nc.scalar.square(sq[:], ce[:])
                        # q via halving tree (2x-packed adds beat 1x reduce)
                        sqv = sq.rearrange("p (x d) -> p x d", d=D)
                        qt1 = b_sm.tile([P, Lg * R * 16], BF16, tag="qt1")
                        qt1v = qt1.rearrange("p (x d) -> p x d", d=16)
                        nc.vector.tensor_add(qt1v, sqv[:, :, 0:16],
                                             sqv[:, :, 16:32])
                        qt2 = b_sm.tile([P, Lg * R * 8], BF16, tag="qt2")
                        qt2v = qt2.rearrange("p (x d) -> p x d", d=8)
                        nc.vector.tensor_add(qt2v, qt1v[:, :, 0:8],
                                             qt1v[:, :, 8:16])
                        qt3 = b_sm.tile([P, Lg * R * 4], BF16, tag="qt3")
                        qt3v = qt3.rearrange("p (x d) -> p x d", d=4)
                        nc.vector.tensor_add(qt3v, qt2v[:, :, 0:4],
                                             qt2v[:, :, 4:8])
                        qt4 = b_sm.tile([P, Lg * R * 2], BF16, tag="qt4")
                        qt4v = qt4.rearrange("p (x d) -> p x d", d=2)
                        nc.vector.tensor_add(qt4v, qt3v[:, :, 0:2],
                                             qt3v[:, :, 2:4])
                        q = b_sm.tile([P, Lg * R], F32, tag="q")
                        nc.vector.tensor_add(
                            q[:].rearrange("p (x o) -> p x o", o=1),
                            qt4v[:, :, 0:1], qt4v[:, :, 1:2])
                        dq = b_sm.tile([P, Lg * R], F32, tag="dq")
                        nc.scalar.activation(dq[:], q[:],
                                             mybir.ActivationFunctionType.Sqrt,
                                             bias=eps_s[:])
                        rqi = b_sm.tile([P, Lg * R], F32, tag="rqi")
                        nc.vector.reciprocal_approx_fast(rqi[:], dq[:])
                        # s_w pairs = rbf * (1/|ce_r|), replicated x2 from host
                        s_w2 = b_sm.tile([P, Lg * R * 2], BF16, tag="s_w2")
                        nc.vector.tensor_tensor(
                            out=s_w2.rearrange("p (x b) -> p x b", b=2),
                            in0=rqi[:].rearrange("p (x o) -> p x o", o=1)
                                .to_broadcast([P, Lg * R, 2]),
                            in1=rbf2.rearrange("p (x b) -> p x b", b=2)
                                [:, c0 * R:c1 * R, :],
                            op=mybir.AluOpType.mult)

                        # sce = ce * s_w (broadcast over d in pairs)
                        sce = b_one.tile([P, Lg * CD], BF16, tag="sce")
                        nc.vector.tensor_tensor(
                            out=sce.rearrange("p (x a b) -> p x a b",
                                              a=D // 2, b=2),
                            in0=ce.rearrange("p (x a b) -> p x a b",
                                             a=D // 2, b=2),
                            in1=s_w2.rearrange("p (x o b) -> p x o b", o=1, b=2)
                                .to_broadcast([P, Lg * R, D // 2, 2]),
                            op=mybir.AluOpType.mult)

                        # w = sum_r sce  (halving tree over r)
                        scev = sce.rearrange("p (l r d) -> p l r d", r=R, d=D)
                        t1 = b_sm.tile([P, Lg * (R // 2) * D], BF16, tag="t1")
                        t1v = t1.rearrange("p (l r d) -> p l r d", r=R // 2, d=D)
                        nc.vector.tensor_add(t1v, scev[:, :, 0:R // 2, :],
                                             scev[:, :, R // 2:R, :])
                        t2 = b_sm.tile([P, Lg * (R // 4) * D], BF16, tag="t2")
                        t2v = t2.rearrange("p (l r d) -> p l r d", r=R // 4, d=D)
                        nc.vector.tensor_add(t2v, t1v[:, :, 0:R // 4, :],
                                             t1v[:, :, R // 4:R // 2, :])
                        wv = b_sm.tile([P, Lg * D], BF16, tag="wv")
                        wvv = wv.rearrange("p (l d) -> p l d", d=D)
                        nc.vector.tensor_add(wvv, t2v[:, :, 0, :],
                                             t2v[:, :, 1, :])

                        # second l2norm over d
                        wsq = b_sm.tile([P, Lg * D], BF16, tag="wsq")
                        nc.scalar.square(wsq[:], wv[:])
                        ws = b_sm.tile([P, Lg], F32, tag="ws")
                        nc.vector.reduce_sum(
                            ws[:], wsq.rearrange("p (l d) -> p l d", d=D),
                            axis=mybir.AxisListType.X)
                        dw = b_sm.tile([P, Lg], F32, tag="dw")
                        nc.scalar.activation(dw[:], ws[:],
                                             mybir.ActivationFunctionType.Sqrt,
                                             bias=eps_s[:])
                        rwi = b_sm.tile([P, Lg], F32, tag="rqi")
                        nc.vector.reciprocal_approx_fast(rwi[:], dw[:])
                        rw2 = b_sm.tile([P, Lg * 2], BF16, tag="s_w2")
                        nc.vector.tensor_copy(
                            rw2.rearrange("p (l b) -> p l b", b=2),
                            rwi[:].rearrange("p (l o) -> p l o", o=1)
                                .to_broadcast([P, Lg, 2]))

                        # msg = h[dst] * w * rw
                        m1 = b_sm.tile([P, Lg * D], BF16, tag="t1")
                        nc.vector.tensor_tensor(
                            out=m1.rearrange("p (l d) -> p l d", d=D),
                            in0=wvv, in1=td[:, :, CD:TD],
                            op=mybir.AluOpType.mult)
                        msg = b_sm.tile([P, Lg * D], BF16, tag="qt1")
                        nc.vector.tensor_tensor(
                            out=msg.rearrange("p (x a b) -> p x a b",
                                              a=D // 2, b=2),
                            in0=m1.rearrange("p (x a b) -> p x a b",
                                             a=D // 2, b=2),
                            in1=rw2.rearrange("p (x o b) -> p x o b", o=1, b=2)
                                .to_broadcast([P, Lg, D // 2, 2]),
                            op=mybir.AluOpType.mult)

                        # agg[p, d] = sum_l msg  (strided free-axis reduce)
                        aggs = b_out.tile([P, D], BF16, tag="aggs")
                        with nc.allow_low_precision("single-rounding bf16 out"):
                            nc.vector.reduce_sum(
                                aggs[:], msg.rearrange("p (l d) -> p d l", d=D),
                                axis=mybir.AxisListType.X)
                        if nseg > 1:
                            if s == 0:
                                nc.vector.tensor_copy(accf[:], aggs[:])
                            else:
                                nc.vector.tensor_add(accf[:], accf[:], aggs[:])

                    agg = accf if nseg > 1 else aggs

                    # out rows = agg @ Wu
                    aggTp = b_ps_t.tile([D, P], BF16, tag="aggTp")
                    nc.tensor.transpose(aggTp[:], agg[:], ident[:])
                    aggT = b_out.tile([D, P], BF16, tag="aggT")
                    nc.scalar.copy(aggT[:], aggTp[:])
                    outp = b_ps_f.tile([P, H], F32, tag="outp")
                    nc.tensor.matmul(outp[:wn, :], lhsT=aggT[:, :wn], rhs=Wu_s[:],
                                     start=True, stop=True)
                    outs = b_out.tile([P, H], F32, tag="outs")
                    nc.scalar.copy(outs[:wn, :], outp[:wn, :])
                    nc.sync.dma_start(out[ch * P:ch * P + wn, :], outs[:wn, :])

    nc.finalize()
    return nc


def _silu(v):
    return v * (0.5 * (1.0 + np.tanh(0.5 * v)))


def _prepare(inputs, NSH, H, D, C, R):
    """Host-side sharding: returns (in_maps, LAs, LBs, orders)."""
    x = np.asarray(inputs["x"], np.float32)
    rbfs = np.asarray(inputs["rbfs"], np.float32)
    coeffs = np.asarray(inputs["coeffs"], np.float32)
    W1 = np.asarray(inputs["W1"], np.float32)
    b1 = np.asarray(inputs["b1"], np.float32)
    W2 = np.asarray(inputs["W2"], np.float32)
    b2 = np.asarray(inputs["b2"], np.float32)
    Wc1 = np.asarray(inputs["Wc1"], np.float32)
    Wc2 = np.asarray(inputs["Wc2"], np.float32)
    Wu = np.asarray(inputs["Wu"], np.float32)
    ei = np.asarray(inputs["edge_index"], np.int64)
    src, dst = ei[0], ei[1]
    N, E = x.shape[0], src.shape[0]
    NH2 = N // 2
    n_chunks = (NSH + P - 1) // P
    PADN = n_chunks * P - NSH

    # ---- node -> core deal + per-core ordering (host-side scheduling) ----
    # Deal nodes across cores by (deg, acnt) rank so all 8 cores see nearly
    # identical per-position (a, b) stats, then order each core's nodes by
    # deg desc with a boustrophedon acnt tie-break so each 128-node chunk
    # has tight maxima of both the A-half and B-half slot counts (the
    # per-chunk gather/compute width is the max over all cores and rows).
    deg_g = np.bincount(src, minlength=N)
    node_core = (np.arange(N) // NSH).astype(np.int64)
    for _ in range(3):
        a_g = np.bincount(src[node_core[dst] < (NC // 2)], minlength=N)
        g_order = np.lexsort((-a_g, -deg_g))
        node_core = np.empty(N, np.int64)
        node_core[g_order] = np.arange(N) % NC
    a_g = np.bincount(src[node_core[dst] < (NC // 2)], minlength=N)

    nodes_of = np.empty((NC, NSH), np.int64)   # (core, pos) -> global node
    node_pos = np.empty(N, np.int64)           # global node -> pos in core
    for k in range(NC):
        nodes = np.where(node_core == k)[0]
        assert len(nodes) == NSH
        s = deg_g[nodes]
        key2 = np.where(s % 2 == 0, -a_g[nodes], a_g[nodes])
        o = nodes[np.lexsort((key2, -s))]
        nodes_of[k] = o
        node_pos[o] = np.arange(NSH)

    core_of = node_core[src]
    d_core = node_core[dst]
    a_flag = d_core < (NC // 2)          # table half of the dst row

    deg = np.zeros((NC, NSH), np.int64)
    np.add.at(deg, (core_of, node_pos[src]), 1)
    acnt = np.zeros((NC, NSH), np.int64)
    np.add.at(acnt, (core_of[a_flag], node_pos[src[a_flag]]), 1)

    pos = node_pos[src]
    ch = pos // P
    prow = pos % P

    # per-chunk slot widths (max over cores and chunk rows); deg/acnt are
    # already in (core, pos) layout
    def chunk_max(v):
        pad = np.zeros((NC, PADN), np.int64)
        vv = np.concatenate([v, pad], axis=1).reshape(NC, n_chunks, P)
        return vv.max(axis=(0, 2))
    bcnt = deg - acnt
    LAs = np.maximum(chunk_max(acnt), 1).astype(np.int64)
    LBs = np.maximum(chunk_max(bcnt), 1).astype(np.int64)
    Ls = LAs + LBs
    offs = np.concatenate(([0], np.cumsum(Ls)))
    SLOT = int(offs[-1])

    # l-index of each edge within its (core, node, half) list
    key = (core_of * NSH + pos) * 2 + (~a_flag).astype(np.int64)
    order_e = np.argsort(key, kind="stable")
    key_sorted = key[order_e]
    starts = np.concatenate(([0], np.cumsum(np.bincount(
        key_sorted, minlength=NC * NSH * 2))))
    lidx_sorted = np.arange(E) - starts[key_sorted]
    lidx = np.empty(E, np.int64)
    lidx[order_e] = lidx_sorted

    # dst -> permuted row id within its table half (int16-safe)
    gid = d_core * NSH + node_pos[dst]
    hid = np.where(a_flag, gid, gid - NH2)

    col = offs[ch] + np.where(a_flag, lidx, LAs[ch] + lidx)
    idx_all = np.zeros((NC, P, SLOT), np.int16)
    rbf_all = np.zeros((NC, P, SLOT, R), np.float32)
    idx_all[core_of, prow, col] = hid.astype(np.int16)
    rbf_all[core_of, prow, col] = rbfs

    # wrap indices for dma_gather: per slot column l, wrapped cols 8l..8l+7
    # hold idxvec[l*128 + p] at [p % 16, 8l + p//16], replicated over the
    # eight 16-partition groups.
    idx_w = np.zeros((NC, P, 8 * SLOT), np.int16)
    iv = idx_all.transpose(0, 2, 1).reshape(NC, SLOT * P)   # [NC, l-major]
    w16 = iv.reshape(NC, SLOT * 8, 16).transpose(0, 2, 1)   # [NC, 16, 8*SLOT]
    idx_w[:, :, :] = np.tile(w16, (1, 8, 1))

    sx = _silu(x)
    sc = _silu(coeffs)

    in_maps = []
    for k in range(NC):
        o = nodes_of[k]
        in_maps.append({
            "sxT": np.ascontiguousarray(sx[o].T.astype(bfloat16)),
            # (r, n)-major columns: tile j of the coeffs MLP covers one r
            "scT": np.ascontiguousarray(
                sc[o].transpose(1, 0, 2).reshape(R * NSH, C).T
                .astype(bfloat16)),
            "W1b": np.ascontiguousarray(W1.astype(bfloat16)),
            "b1": np.ascontiguousarray(b1.reshape(H, 1)),
            "W2b": np.ascontiguousarray(W2.astype(bfloat16)),
            "b2r": np.ascontiguousarray(np.tile(b2, (P, 8)).astype(np.float32)),
            "Wc1b": np.ascontiguousarray(Wc1.astype(bfloat16)),
            "Wc2b": np.ascontiguousarray(Wc2.astype(bfloat16)),
            "Wub": np.ascontiguousarray(Wu.astype(bfloat16)),
            "idxW": np.ascontiguousarray(idx_w[k]),
            "rbfE": np.ascontiguousarray(
                np.repeat(rbf_all[k].reshape(P, SLOT * R), 2, axis=1)
                .astype(bfloat16)),
        })
    return (in_maps, tuple(int(v) for v in LAs), tuple(int(v) for v in LBs),
            nodes_of)


_CACHE = {}


def run(inputs, trace=False):
    """Returns (output, BassKernelResults)."""
    x = np.asarray(inputs["x"])
    coeffs = np.asarray(inputs["coeffs"])
    N, H = x.shape
    _, R, C = coeffs.shape
    D = np.asarray(inputs["W2"]).shape[1]
    assert N % NC == 0
    NSH = N // NC

    in_maps, LAs, LBs, nodes_of = _prepare(inputs, NSH, H, D, C, R)
    key = (NSH, H, D, C, R, LAs, LBs)
    if key not in _CACHE:
        _CACHE[key] = _build(NSH, H, D, C, R, LAs, LBs)
    nc = _CACHE[key]
    res = run_bass_kernel_spmd(nc, in_maps, core_ids=list(range(NC)),
                               trace=trace)
    full = np.empty((N, H), np.float32)
    for k in range(NC):
        full[nodes_of[k]] = res.results[k]["out"]
    return full, res


def kernel(**inputs) -> np.ndarray:
    out, _ = run(inputs, trace=False)
    return out

